# revision 1
# baseline (speedup 1.0000x reference)
"""Compact Bilinear Pooling (count-sketch + FFT circular correlation) as a
Trainium2 Bass kernel, data-parallel over batch across 8 NeuronCores.

Math: FFT(count_sketch(x; s, h))[k] = sum_c x[c] * s[c] * exp(-2pi i h[c] k / D)
    = x @ A, a dense complex matrix built on the host from (s, h). So the whole
layer is: Y1 = X1 @ A1, Y2 = X2 @ A2 (per-row half spectra), elementwise
complex product, sum-pool over the 14x14 window (via a 0/1 pooling matmul),
then a real inverse FFT of the pooled [4, D] spectrum per core, done as a
two-stage Cooley-Tukey factorization (D = 64*128) of small matmuls.

All matmuls run as float32r (TF32-like, 1 cycle/row on the PE).
"""
import numpy as np

import concourse.bass as bass
import concourse.tile as tile
from concourse import bacc, mybir
from concourse.bass_utils import run_bass_kernel_spmd

B, Hh, Ww, C, D = 32, 14, 14, 512, 8192
NCORES = 8
BPC = B // NCORES        # 4 batches per core
HW = Hh * Ww             # 196
ROWS = BPC * HW          # 784 rows per core
RT = 7                   # row tiles of 128
ROWS_PAD = RT * 128      # 896
KT = 33                  # frequency tiles of 128
KP = KT * 128            # 4224 >= D/2 + 1
CCN = 4                  # contraction chunks (C = 4*128)

F32 = mybir.dt.float32
F32R = mybir.dt.float32r


def _round_fp32r(x: np.ndarray) -> np.ndarray:
    """Round-to-nearest-even dropping the low 12 mantissa bits (measured
    float32r behaviour of the DVE rounding path on trn2)."""
    b = np.ascontiguousarray(x, dtype=np.float32).view(np.uint32)
    r = (b + np.uint32(0x7FF) + ((b >> np.uint32(12)) & np.uint32(1))) & np.uint32(0xFFFFF000)
    return r.view(np.float32)


def _build_nc():
    nc = bacc.Bacc("TRN2", target_bir_lowering=False)

    xt_d = nc.dram_tensor("xt", [128, 2, CCN, ROWS], F32R, kind="ExternalInput")
    amat_d = nc.dram_tensor("amat", [128, 4, CCN, KT, 128], F32R, kind="ExternalInput")
    w1_d = nc.dram_tensor("w1", [128, 3, 128], F32R, kind="ExternalInput")
    w2_d = nc.dram_tensor("w2", [64, 2, 64], F32R, kind="ExternalInput")
    tw_d = nc.dram_tensor("tw", [64, 2, 128], F32, kind="ExternalInput")
    id_d = nc.dram_tensor("ident", [128, 128], F32, kind="ExternalInput")
    out_d = nc.dram_tensor("out", [BPC, D], F32, kind="ExternalOutput")

    with tile.TileContext(nc) as tc:
        with tc.tile_pool(name="const", bufs=1) as pc, \
             tc.tile_pool(name="astream", bufs=2) as pa, \
             tc.tile_pool(name="work", bufs=2) as pw, \
             tc.tile_pool(name="qstage", bufs=2) as pqs, \
             tc.tile_pool(name="dram", bufs=1, space="DRAM") as pd:

            xt = pc.tile([128, 2, CCN, ROWS], F32R)
            for inp in range(2):
                for cc in range(CCN):
                    nc.sync.dma_start(xt[:, inp, cc], xt_d[:, inp, cc])
            qsb = pc.tile([128, KT, 4, 4], F32)

            qdram = pd.tile([8, D], F32R)
            # zero-fill the padded tail of the spectrum
            zs = pc.tile([8, D - KP], F32)
            nc.vector.memset(zs, 0.0)
            nc.sync.dma_start(qdram[:, KP:], zs.bitcast(F32R))

            # ---------------- main loop ----------------
            with tc.tile_pool(name="py", bufs=2, space="PSUM") as py:
                for kt in range(KT):
                    at = pa.tile([128, 4, CCN, 128], F32R, tag="amat")
                    for t_ in range(4):
                        nc.sync.dma_start(at[:, t_], amat_d[:, t_, :, kt, :])
                    for rc in range(2):
                        yps = {}
                        for t in range(4):
                            yps[t] = py.tile([128, 392], F32, tag=f"y{t}", name=f"y{t}")
                        for cc in range(CCN):
                            for inp in range(2):
                                for ri in range(2):
                                    t = inp * 2 + ri
                                    nc.tensor.matmul(
                                        yps[t],
                                        at[:, t, cc, :],
                                        xt[:, inp, cc, rc * 392:(rc + 1) * 392],
                                        start=(cc == 0),
                                        stop=(cc == CCN - 1),
                                    )
                        # copy Y1r/Y1i to SBUF; Y2r/Y2i stay in PSUM
                        ysb = {}
                        for t in range(2):
                            ysb[t] = pw.tile([128, 392], F32, tag=f"ysb{t}", name=f"ysb{t}")
                            nc.scalar.copy(ysb[t], yps[t])
                        # products (DVE, one PSUM operand each)
                        # (0: U=y1r*y2r, 1: V=y1i*y2i, 2: T1=y1r*y2i, 3: T2=y1i*y2r)
                        prods = {}
                        for term, (i0, i1) in enumerate([(0, 2), (1, 3), (0, 3), (1, 2)]):
                            prods[term] = pw.tile([128, 392], F32, tag=f"prod{term}",
                                                  name=f"prod{term}")
                            nc.vector.tensor_mul(prods[term], ysb[i0], yps[i1])
                        # pooling: per-batch free-axis sums.
                        # term 0 on DVE (multi-output reduce), terms 1-3 on ACT
                        # via activation Copy + accum_out.
                        nc.vector.tensor_reduce(
                            qsb[:, kt, 0, rc * 2:(rc + 1) * 2],
                            prods[0].rearrange("p (s x) -> p s x", s=2),
                            axis=mybir.AxisListType.X, op=mybir.AluOpType.add)
                        act_scr = pw.tile([128, 196], F32, tag="act_scr")
                        for term in range(1, 4):
                            for seg in range(2):
                                b = rc * 2 + seg
                                nc.scalar.activation(
                                    act_scr, prods[term][:, seg * 196:(seg + 1) * 196],
                                    mybir.ActivationFunctionType.Copy,
                                    accum_out=qsb[:, kt, term, b:b + 1])
                # combine terms: Qr = U - V, Qi = T1 + T2 (round to f32r),
                # laid out as [p, kt, {r,i}, b] = [p, kt*8]
                qstf = pqs.tile([128, KT, 2, 4], F32R, tag="qstf")
                nc.vector.tensor_sub(qstf[:, :, 0, :], qsb[:, :, 0, :], qsb[:, :, 1, :])
                nc.vector.tensor_add(qstf[:, :, 1, :], qsb[:, :, 2, :], qsb[:, :, 3, :])
                qstf_v = qstf.rearrange("p kt i b -> p kt (i b)")
                for j in range(8):
                    nc.sync.dma_start(
                        qdram[j, :KP].rearrange("(kt p) -> p kt", p=128),
                        qstf_v[:, :, j])

            # ---------------- inverse FFT tail ----------------
            w1 = pc.tile([128, 3, 128], F32R)
            nc.sync.dma_start(w1, w1_d[:, :, :])
            w2 = pc.tile([64, 2, 64], F32R)
            nc.sync.dma_start(w2, w2_d[:, :, :])
            tw = pc.tile([64, 2, 128], F32)
            nc.sync.dma_start(tw, tw_d[:, :, :])
            ident = pc.tile([128, 128], F32)
            nc.sync.dma_start(ident, id_d[:, :])

            with tc.tile_pool(name="pif", bufs=1, space="PSUM") as pif, \
                 tc.tile_pool(name="ptr", bufs=2, space="PSUM") as ptr, \
                 tc.tile_pool(name="pifs", bufs=1) as pifs, \
                 tc.tile_pool(name="ptmp", bufs=2) as ptmp:
                # reshape DMA: Qc as [a=128, b=4, r=64]
                qar = pifs.tile([128, BPC * 64], F32R, tag="qar")
                qai = pifs.tile([128, BPC * 64], F32R, tag="qai")
                nc.sync.dma_start(qar.rearrange("p (b r) -> p b r", r=64),
                                  qdram[0:BPC].rearrange("b (a r) -> a b r", r=64))
                nc.sync.dma_start(qai.rearrange("p (b r) -> p b r", r=64),
                                  qdram[BPC:2 * BPC].rearrange("b (a r) -> a b r", r=64))

                # stage 1: V[q, (b,r)] ; contraction over a
                vr_ps = pif.tile([128, BPC * 64], F32, tag="vr")
                vi_ps = pif.tile([128, BPC * 64], F32, tag="vi")
                nc.tensor.matmul(vr_ps, w1[:, 0, :], qar, start=True, stop=False)
                nc.tensor.matmul(vr_ps, w1[:, 2, :], qai, start=False, stop=True)
                nc.tensor.matmul(vi_ps, w1[:, 1, :], qar, start=True, stop=False)
                nc.tensor.matmul(vi_ps, w1[:, 0, :], qai, start=False, stop=True)
                vr_sb = pifs.tile([128, BPC * 64], F32, tag="vr_sb")
                vi_sb = pifs.tile([128, BPC * 64], F32, tag="vi_sb")
                nc.scalar.copy(vr_sb, vr_ps)
                nc.scalar.copy(vi_sb, vi_ps)

                # per-b transpose [128 q, 64 r] -> [64 r, 128 q], then twiddle
                tr_sb = pifs.tile([64, BPC * 128], F32R, tag="tr_sb")
                ti_sb = pifs.tile([64, BPC * 128], F32R, tag="ti_sb")
                for b in range(BPC):
                    trp = ptr.tile([64, 128], F32, tag="trp")
                    tip = ptr.tile([64, 128], F32, tag="tip")
                    nc.tensor.transpose(trp, vr_sb[:, b * 64:(b + 1) * 64], ident)
                    nc.tensor.transpose(tip, vi_sb[:, b * 64:(b + 1) * 64], ident)
                    m1 = ptmp.tile([64, 128], F32, tag="m1")
                    m2 = ptmp.tile([64, 128], F32, tag="m2")
                    m3 = ptmp.tile([64, 128], F32, tag="m3")
                    m4 = ptmp.tile([64, 128], F32, tag="m4")
                    nc.vector.tensor_mul(m1, trp, tw[:, 0, :])
                    nc.vector.tensor_mul(m2, tip, tw[:, 1, :])
                    nc.vector.tensor_mul(m3, trp, tw[:, 1, :])
                    nc.vector.tensor_mul(m4, tip, tw[:, 0, :])
                    nc.vector.tensor_sub(tr_sb[:, b * 128:(b + 1) * 128], m1, m2)
                    nc.vector.tensor_add(ti_sb[:, b * 128:(b + 1) * 128], m3, m4)

                # stage 2: out[t, (b,q)] = c2^T Tr + (-s2)^T Ti
                ops = pif.tile([64, BPC * 128], F32, tag="ops")
                nc.tensor.matmul(ops, w2[:, 0, :], tr_sb, start=True, stop=False)
                nc.tensor.matmul(ops, w2[:, 1, :], ti_sb, start=False, stop=True)
                res = pifs.tile([64, BPC * 128], F32, tag="res")
                nc.scalar.copy(res, ops)
                for b in range(BPC):
                    nc.sync.dma_start(
                        out_d[b].rearrange("(t q) -> t q", q=128),
                        res[:, b * 128:(b + 1) * 128])

    nc.compile()
    return nc


def _host_consts(rand_s_1, rand_s_2, rand_h_1, rand_h_2):
    k = np.arange(KP)
    alpha = np.where((k == 0) | (k == D // 2), 1.0, 2.0) / D
    alpha = np.where(k > D // 2, 0.0, alpha)
    live = (k <= D // 2).astype(np.float64)
    s1 = rand_s_1.astype(np.float64)
    s2 = rand_s_2.astype(np.float64)
    th1 = 2.0 * np.pi * ((rand_h_1.astype(np.int64)[:, None] * k[None, :]) % D) / D
    th2 = 2.0 * np.pi * ((rand_h_2.astype(np.int64)[:, None] * k[None, :]) % D) / D
    A = np.empty((4, C, KP), np.float32)
    A[0] = s1[:, None] * np.cos(th1) * alpha
    A[1] = -s1[:, None] * np.sin(th1) * alpha
    A[2] = s2[:, None] * np.cos(th2) * live
    A[3] = -s2[:, None] * np.sin(th2) * live
    # amat layout [p, tensor, cc, kt, 128]
    amat = np.ascontiguousarray(
        A.reshape(4, CCN, 128, KT, 128).transpose(2, 0, 1, 3, 4))
    amat = _round_fp32r(amat)

    a = np.arange(128)[:, None]
    q = np.arange(128)[None, :]
    c1 = np.cos(2 * np.pi * a * q / 128)
    s1m = np.sin(2 * np.pi * a * q / 128)
    w1 = np.stack([c1, s1m, -s1m], 1).astype(np.float32)  # [128, 3, 128]
    r_ = np.arange(64)[:, None]
    t_ = np.arange(64)[None, :]
    c2 = np.cos(2 * np.pi * t_ * r_ / 64)
    s2m = np.sin(2 * np.pi * t_ * r_ / 64)
    w2 = np.stack([c2, -s2m], 1).astype(np.float32)       # [64, 2, 64]
    ctw = np.cos(2 * np.pi * q * r_ / D)
    stw = np.sin(2 * np.pi * q * r_ / D)
    tw = np.stack([ctw, stw], 1).astype(np.float32)       # [64, 2, 128]
    ident = np.eye(128, dtype=np.float32)
    return amat, _round_fp32r(w1), _round_fp32r(w2), tw, ident


_NC_CACHE = None
LAST_RESULTS = None


def kernel(bottom1, bottom2, rand_s_1, rand_s_2, rand_h_1, rand_h_2):
    global _NC_CACHE
    if _NC_CACHE is None:
        _NC_CACHE = _build_nc()
    nc = _NC_CACHE

    amat, w1, w2, tw, ident = _host_consts(
        np.asarray(rand_s_1), np.asarray(rand_s_2),
        np.asarray(rand_h_1), np.asarray(rand_h_2))

    x1 = np.asarray(bottom1, np.float32).reshape(B, HW, C)
    x2 = np.asarray(bottom2, np.float32).reshape(B, HW, C)

    in_maps = []
    for core in range(NCORES):
        bs = slice(core * BPC, (core + 1) * BPC)
        xt = np.empty((2, C, ROWS), np.float32)
        xt[0] = x1[bs].reshape(ROWS, C).T
        xt[1] = x2[bs].reshape(ROWS, C).T
        xt = np.ascontiguousarray(
            xt.reshape(2, CCN, 128, ROWS).transpose(2, 0, 1, 3))
        xt = _round_fp32r(xt)
        in_maps.append({
            "xt": xt, "amat": amat,
            "w1": w1, "w2": w2, "tw": tw, "ident": ident,
        })

    res = run_bass_kernel_spmd(nc, in_maps, core_ids=list(range(NCORES)))
    global LAST_RESULTS
    LAST_RESULTS = res
    out = np.concatenate([res.results[c]["out"] for c in range(NCORES)], 0)
    return out.astype(np.float32)


if __name__ == "__main__":
    rng = np.random.default_rng(0)
    b1 = rng.standard_normal((B, Hh, Ww, C)).astype(np.float32)
    b2 = rng.standard_normal((B, Hh, Ww, C)).astype(np.float32)
    s1 = (2.0 * rng.integers(0, 2, C) - 1.0).astype(np.float32)
    s2 = (2.0 * rng.integers(0, 2, C) - 1.0).astype(np.float32)
    h1 = rng.integers(0, D, C, dtype=np.int32)
    h2 = rng.integers(0, D, C, dtype=np.int32)
    out = kernel(bottom1=b1, bottom2=b2, rand_s_1=s1, rand_s_2=s2,
                 rand_h_1=h1, rand_h_2=h2)
    print(out.shape, out.dtype)



# revision 5
# speedup vs baseline: 1.5750x; 1.5750x over previous
"""Compact Bilinear Pooling (count-sketch + FFT circular correlation) as a
Trainium2 Bass kernel, data-parallel over batch across 8 NeuronCores.

Math: FFT(count_sketch(x; s, h))[k] = sum_c x[c] * s[c] * exp(-2pi i h[c] k / D)
    = x @ A, a dense complex matrix built on the host from (s, h). So the whole
layer is: Y1 = X1 @ A1, Y2 = X2 @ A2 (per-row half spectra, fp16 matmuls),
elementwise complex product (fp16 on DVE), sum-pool over the 14x14 window
(pairwise fp16 fold on the Pool engine + fp32 free-axis reduce on DVE), then a
real inverse FFT of the pooled [4, D] spectrum per core, done fully on-chip as
a two-stage Cooley-Tukey factorization (D = 64*128, k = kt*128 + p) built from
PE transposes + small matmuls -- no DRAM round trip.
"""
import numpy as np

import concourse.bass as bass
import concourse.tile as tile
from concourse import bacc, mybir
from concourse.bass_utils import run_bass_kernel_spmd

B, Hh, Ww, C, D = 32, 14, 14, 512, 8192
NCORES = 8
BPC = B // NCORES        # 4 batches per core
HW = Hh * Ww             # 196
ROWS = BPC * HW          # 784 rows per core
KT = 33                  # frequency tiles of 128 (KT*128 = 4224 >= D/2 + 1)
KP = KT * 128
CCN = 4                  # contraction chunks (C = 4*128)

F32 = mybir.dt.float32
F32R = mybir.dt.float32r
F16 = mybir.dt.float16


def _round_fp32r(x: np.ndarray) -> np.ndarray:
    """Round-to-nearest-even dropping the low 12 mantissa bits (measured
    float32r behaviour of the DVE rounding path on trn2)."""
    b = np.ascontiguousarray(x, dtype=np.float32).view(np.uint32)
    r = (b + np.uint32(0x7FF) + ((b >> np.uint32(12)) & np.uint32(1))) & np.uint32(0xFFFFF000)
    return r.view(np.float32)


def _build_nc():
    nc = bacc.Bacc("TRN2", target_bir_lowering=False)

    xt_d = nc.dram_tensor("xt", [128, 2, CCN, ROWS], F16, kind="ExternalInput")
    amat_d = nc.dram_tensor("amat", [128, KT, 4, CCN, 128], F16, kind="ExternalInput")
    c33_d = nc.dram_tensor("c33", [KT, 3, 64], F32R, kind="ExternalInput")
    tw_d = nc.dram_tensor("tw", [128, 2, 64], F32, kind="ExternalInput")
    w2_d = nc.dram_tensor("w2", [128, 2, 128], F32R, kind="ExternalInput")
    id_d = nc.dram_tensor("ident", [128, 128], F32, kind="ExternalInput")
    out_d = nc.dram_tensor("out", [BPC, D], F32, kind="ExternalOutput")

    with tile.TileContext(nc) as tc:
        with tc.tile_pool(name="const", bufs=1) as pc, \
             tc.tile_pool(name="astream", bufs=3) as pa, \
             tc.tile_pool(name="ywork", bufs=2) as pyb, \
             tc.tile_pool(name="pwork", bufs=2) as pp:

            xt = pc.tile([128, 2, CCN, ROWS], F16)
            nc.sync.dma_start(xt, xt_d[:, :, :, :])
            c33 = pc.tile([KT, 3, 64], F32R)
            nc.sync.dma_start(c33, c33_d[:, :, :])
            tw = pc.tile([128, 2, 64], F32)
            nc.sync.dma_start(tw, tw_d[:, :, :])
            w2 = pc.tile([128, 2, 128], F32R)
            nc.sync.dma_start(w2, w2_d[:, :, :])
            ident = pc.tile([128, 128], F32)
            nc.sync.dma_start(ident, id_d[:, :])

            # pooled spectrum accumulator [p, kt, term, b]
            qsb = pc.tile([128, KT, 4, 4], F32)

            # ---------------- main loop ----------------
            with tc.tile_pool(name="py", bufs=2, space="PSUM") as py:
                for kt in range(KT):
                    at = pa.tile([128, 4, CCN, 128], F16, tag="amat")
                    nc.sync.dma_start(at, amat_d[:, kt])
                    for rc in range(2):
                        yps = {}
                        for t in range(4):
                            yps[t] = py.tile([128, 392], F32, tag=f"y{t}", name=f"y{t}")
                        for cc in range(CCN):
                            for t in range(4):
                                nc.tensor.matmul(
                                    yps[t],
                                    at[:, t, cc],
                                    xt[:, t // 2, cc, rc * 392:(rc + 1) * 392],
                                    start=(cc == 0),
                                    stop=(cc == CCN - 1),
                                )
                        # PSUM -> SBUF fp16 copies (ACT)
                        ysb = pyb.tile([128, 4, 392], F16, tag="ysb")
                        for t in range(4):
                            nc.scalar.copy(ysb[:, t], yps[t])
                        # products (DVE, all fp16 SBUF -> 2x mode)
                        # (0: U=y1r*y2r, 1: V=y1i*y2i, 2: T1=y1r*y2i, 3: T2=y1i*y2r)
                        prod = pp.tile([128, 4, 2, 196], F16, tag="prod")
                        for term, (i0, i1) in enumerate(((0, 2), (1, 3), (0, 3), (1, 2))):
                            nc.vector.tensor_mul(
                                prod[:, term].rearrange("p s x -> p (s x)"),
                                ysb[:, i0], ysb[:, i1])
                        # pairwise fold 196 -> 98 (Pool engine), then per-batch
                        # free-axis sum on DVE in fp32
                        prodF = pp.tile([128, 4, 2, 98], F16, tag="prodF")
                        nc.gpsimd.tensor_add(
                            prodF.rearrange("p t s x -> p (t s) x"),
                            prod[:, :, :, 0:98].rearrange("p t s x -> p (t s) x"),
                            prod[:, :, :, 98:196].rearrange("p t s x -> p (t s) x"))
                        nc.vector.tensor_reduce(
                            qsb[:, kt, :, rc * 2:(rc + 1) * 2],
                            prodF.rearrange("p t s x -> p (t s) x"),
                            axis=mybir.AxisListType.X, op=mybir.AluOpType.add)

            # ---------------- inverse FFT tail (on-chip) ----------------
            # Q[k] with k = kt*128 + p lives as qsb[p, kt].  IFFT via
            # x[t1 + 64*t2] = sum_p e(p t1/8192) e(p t2/128)
            #                   * sum_kt Q[kt*128+p] e(kt t1/64)
            with tc.tile_pool(name="tsb", bufs=1) as pt, \
                 tc.tile_pool(name="tps", bufs=2, space="PSUM") as pps, \
                 tc.tile_pool(name="tw2", bufs=1, space="PSUM") as ppw, \
                 tc.tile_pool(name="tmm", bufs=2) as pm:
                # combine terms: Qr = U - V, Qi = T1 + T2; layout [p, i, kt, b]
                qc = pt.tile([128, 2, KT, 4], F32)
                nc.vector.tensor_sub(qc[:, 0], qsb[:, :, 0, :], qsb[:, :, 1, :])
                nc.gpsimd.tensor_add(qc[:, 1], qsb[:, :, 2, :], qsb[:, :, 3, :])

                # transpose Q -> [kt, p] per (i, b)
                qt_sb = pt.tile([KT, 2, 4, 128], F32R)
                for i in range(2):
                    for b in range(BPC):
                        qt_ps = pps.tile([KT, 128], F32, tag="qt")
                        nc.tensor.transpose(qt_ps, qc[:, i, :, b], ident)
                        nc.scalar.copy(qt_sb[:, i, b], qt_ps)

                # stage 1: W[t1, p] = sum_kt Q[kt, p] e(kt t1 / 64)
                w_sb = pt.tile([64, 2, 4, 128], F32)
                for b in range(BPC):
                    wr = ppw.tile([64, 128], F32, tag="wr")
                    wi = ppw.tile([64, 128], F32, tag="wi")
                    nc.tensor.matmul(wr, c33[:, 0], qt_sb[:, 0, b], start=True, stop=False)
                    nc.tensor.matmul(wr, c33[:, 2], qt_sb[:, 1, b], start=False, stop=True)
                    nc.tensor.matmul(wi, c33[:, 1], qt_sb[:, 0, b], start=True, stop=False)
                    nc.tensor.matmul(wi, c33[:, 0], qt_sb[:, 1, b], start=False, stop=True)
                    nc.scalar.copy(w_sb[:, 0, b], wr)
                    nc.scalar.copy(w_sb[:, 1, b], wi)

                # transpose W -> [p, t1], twiddle by e(p t1 / 8192)
                g_sb = pt.tile([128, 2, 4, 64], F32R)
                for b in range(BPC):
                    wrt = ppw.tile([128, 64], F32, tag="wrt")
                    wit = ppw.tile([128, 64], F32, tag="wit")
                    nc.tensor.transpose(wrt, w_sb[:, 0, b], ident[:64, :64])
                    nc.tensor.transpose(wit, w_sb[:, 1, b], ident[:64, :64])
                    m1 = pm.tile([128, 64], F32, tag="m1")
                    m2 = pm.tile([128, 64], F32, tag="m2")
                    m3 = pm.tile([128, 64], F32, tag="m3")
                    m4 = pm.tile([128, 64], F32, tag="m4")
                    nc.vector.tensor_mul(m1, wrt, tw[:, 0])
                    nc.vector.tensor_mul(m2, wit, tw[:, 1])
                    nc.vector.tensor_mul(m3, wrt, tw[:, 1])
                    nc.vector.tensor_mul(m4, wit, tw[:, 0])
                    nc.vector.tensor_sub(g_sb[:, 0, b], m1, m2)
                    nc.gpsimd.tensor_add(g_sb[:, 1, b], m3, m4)

                # stage 2: out[t2, b, t1] = sum_p Gr e_c(p t2/128) - Gi e_s(...)
                x_ps = ppw.tile([128, BPC * 64], F32, tag="xps")
                nc.tensor.matmul(x_ps, w2[:, 0], g_sb[:, 0].rearrange("p b t -> p (b t)"),
                                 start=True, stop=False)
                nc.tensor.matmul(x_ps, w2[:, 1], g_sb[:, 1].rearrange("p b t -> p (b t)"),
                                 start=False, stop=True)
                res = pt.tile([128, BPC, 64], F32)
                nc.scalar.copy(res, x_ps)
                nc.sync.dma_start(out_d.rearrange("b (t2 t1) -> t2 b t1", t1=64), res)

    nc.compile()
    return nc


def _host_consts(rand_s_1, rand_s_2, rand_h_1, rand_h_2):
    k = np.arange(KP)
    alpha = np.where((k == 0) | (k == D // 2), 1.0, 2.0) / D
    alpha = np.where(k > D // 2, 0.0, alpha)
    live = (k <= D // 2).astype(np.float64)
    s1 = rand_s_1.astype(np.float64)
    s2 = rand_s_2.astype(np.float64)
    th1 = 2.0 * np.pi * ((rand_h_1.astype(np.int64)[:, None] * k[None, :]) % D) / D
    th2 = 2.0 * np.pi * ((rand_h_2.astype(np.int64)[:, None] * k[None, :]) % D) / D
    A = np.empty((4, C, KP), np.float32)
    A[0] = s1[:, None] * np.cos(th1) * alpha
    A[1] = -s1[:, None] * np.sin(th1) * alpha
    A[2] = s2[:, None] * np.cos(th2) * live
    A[3] = -s2[:, None] * np.sin(th2) * live
    # amat layout [p, kt, tensor, cc, q]: contiguous 4KB per (p, kt)
    amat = np.ascontiguousarray(
        A.reshape(4, CCN, 128, KT, 128).transpose(2, 3, 0, 1, 4)).astype(np.float16)

    kt_ = np.arange(KT)[:, None]
    t1 = np.arange(64)[None, :]
    c_ = np.cos(2 * np.pi * kt_ * t1 / 64)
    s_ = np.sin(2 * np.pi * kt_ * t1 / 64)
    c33 = _round_fp32r(np.stack([c_, s_, -s_], 1).astype(np.float32))  # [KT, 3, 64]

    p_ = np.arange(128)[:, None]
    tw = np.stack([np.cos(2 * np.pi * p_ * t1 / D),
                   np.sin(2 * np.pi * p_ * t1 / D)], 1).astype(np.float32)  # [128, 2, 64]

    t2 = np.arange(128)[None, :]
    w2 = _round_fp32r(np.stack([np.cos(2 * np.pi * p_ * t2 / 128),
                                -np.sin(2 * np.pi * p_ * t2 / 128)],
                               1).astype(np.float32))  # [128, 2, 128]
    ident = np.eye(128, dtype=np.float32)
    return amat, c33, tw, w2, ident


_NC_CACHE = None
LAST_RESULTS = None


def kernel(bottom1, bottom2, rand_s_1, rand_s_2, rand_h_1, rand_h_2):
    global _NC_CACHE
    if _NC_CACHE is None:
        _NC_CACHE = _build_nc()
    nc = _NC_CACHE

    amat, c33, tw, w2, ident = _host_consts(
        np.asarray(rand_s_1), np.asarray(rand_s_2),
        np.asarray(rand_h_1), np.asarray(rand_h_2))

    x1 = np.asarray(bottom1, np.float32).reshape(B, HW, C)
    x2 = np.asarray(bottom2, np.float32).reshape(B, HW, C)

    in_maps = []
    for core in range(NCORES):
        bs = slice(core * BPC, (core + 1) * BPC)
        xt = np.empty((2, C, ROWS), np.float32)
        xt[0] = x1[bs].reshape(ROWS, C).T
        xt[1] = x2[bs].reshape(ROWS, C).T
        xt = np.ascontiguousarray(
            xt.reshape(2, CCN, 128, ROWS).transpose(2, 0, 1, 3)).astype(np.float16)
        in_maps.append({
            "xt": xt, "amat": amat,
            "c33": c33, "tw": tw, "w2": w2, "ident": ident,
        })

    res = run_bass_kernel_spmd(nc, in_maps, core_ids=list(range(NCORES)))
    global LAST_RESULTS
    LAST_RESULTS = res
    out = np.concatenate([res.results[c]["out"] for c in range(NCORES)], 0)
    return out.astype(np.float32)


if __name__ == "__main__":
    rng = np.random.default_rng(0)
    b1 = rng.standard_normal((B, Hh, Ww, C)).astype(np.float32)
    b2 = rng.standard_normal((B, Hh, Ww, C)).astype(np.float32)
    s1 = (2.0 * rng.integers(0, 2, C) - 1.0).astype(np.float32)
    s2 = (2.0 * rng.integers(0, 2, C) - 1.0).astype(np.float32)
    h1 = rng.integers(0, D, C, dtype=np.int32)
    h2 = rng.integers(0, D, C, dtype=np.int32)
    out = kernel(bottom1=b1, bottom2=b2, rand_s_1=s1, rand_s_2=s2,
                 rand_h_1=h1, rand_h_2=h2)
    print(out.shape, out.dtype)
